# revision 8
# baseline (speedup 1.0000x reference)
"""Trainium2 Bass kernel for a causal-attention-like module.

Math (reassociated from the reference nn.Module):
    dist[i,j] = sqrt(max(|T_i|^2 + |T_j|^2 - 2 T_i.T_j, 0) + 1e-8)
    scale_i   = 1 / (1 + mean_j dist[i,j])
    Q2        = (H Wq^T + bq) Wk / sqrt(d)         # bk cancels inside softmax
    E[i,j]    = exp(Q2[i,:] . H[j,:])              # logits bounded ~[-10,10]
    G         = E @ H                              # unnormalized
    out       = ((G / rowsum(E)) Wv^T + bv) * scale @ Wo^T + bo

Sharding: rows of H/T (i dimension) split across 8 cores, 1024 rows each;
H (both orientations) and the small dim x dim weights replicated.

Device layouts per core:
  - distance phase: [i_part, j_free]; |T|^2 terms enter the matmul as two
    extra contraction rows; sqrt+row-accumulate fused on the ACT engine.
  - attention phase: [j_part, i_free]; E tiles feed G^T = sum_j H_j^T E_j
    with natural-layout H tiles stationary, so no on-chip transposes.
All large matmuls run in bf16 (full PE rate) with f32 PSUM accumulation.
"""

import math
import os
import sys

import numpy as np

for _p in ("/opt/trn_rl_repo", "/root/.axon_site", "/root/.axon_site/_ro/trn_rl_repo"):
    if os.path.isdir(_p) and _p not in sys.path:
        sys.path.append(_p)

import ml_dtypes

import concourse.bass as bass
import concourse.mybir as mybir
import concourse.tile as tile
from concourse import bacc, bass_utils

N = 8192          # total rows
D = 512           # feature dim
NCORES = 8
R = N // NCORES   # rows per core (1024)
P = 128           # partitions
KT = D // P       # 4 contraction tiles
CH = 512          # free-dim chunk (one PSUM bank of f32)
NJC = N // CH     # 16 j-chunks
NJT = N // P      # 64 j-tiles
NIC = R // CH     # 2 i-chunks
NIT = R // P      # 8 i-tiles
BF = mybir.dt.bfloat16
F32 = mybir.dt.float32
AF = mybir.ActivationFunctionType
ALU = mybir.AluOpType
INV_SQRT_D = 1.0 / math.sqrt(D)

bf16 = ml_dtypes.bfloat16


def _emit(tc, io):
    nc = tc.nc
    from contextlib import ExitStack

    with ExitStack() as ctx:
        const = ctx.enter_context(tc.tile_pool(name="const", bufs=1))
        psum = ctx.enter_context(tc.tile_pool(name="psum", bufs=1, space="PSUM"))
        dram = ctx.enter_context(tc.tile_pool(name="dram", bufs=1, space="DRAM"))

        # ---- long-lived residents ------------------------------------------
        HT = []
        for k in range(KT):
            ht_t = const.tile([P, N], BF, name=f"ht{k}")
            nc.sync.dma_start(ht_t, io["HTb"][k * P:(k + 1) * P, :])
            HT.append(ht_t)
        WvT, WoT = [], []
        for m in range(KT):
            wvt_t = const.tile([P, D], BF, name=f"wvt{m}")
            nc.sync.dma_start(wvt_t, io["WvTb"][m * P:(m + 1) * P, :])
            WvT.append(wvt_t)
            wot_t = const.tile([P, D], BF, name=f"wot{m}")
            nc.sync.dma_start(wot_t, io["WoTb"][m * P:(m + 1) * P, :])
            WoT.append(wot_t)
        bv_row = const.tile([1, D], BF, name="bvrow")
        nc.sync.dma_start(bv_row, io["bvb"][:, :])
        bo_row = const.tile([1, D], BF, name="borow")
        nc.sync.dma_start(bo_row, io["bob"][:, :])

        ones_p = const.tile([P, 1], BF, name="onesp")
        nc.vector.memset(ones_p, 1.0)
        ones_f1 = const.tile([1, P], F32, name="onesf1")
        nc.vector.memset(ones_f1, 1.0)
        ones_b1 = const.tile([1, P], BF, name="onesb1")
        nc.vector.memset(ones_b1, 1.0)
        eps_col = const.tile([P, 1], F32, name="epscol")
        nc.vector.memset(eps_col, 1e-8)

        Q2T = [const.tile([P, R], BF, name=f"q2t{k}") for k in range(KT)]
        GT = [const.tile([P, R], BF, name=f"gt{d_}") for d_ in range(KT)]
        YT = [const.tile([P, R], BF, name=f"yt{m}") for m in range(KT)]
        SNB = const.tile([P, R], F32, name="snb")
        scl_row = const.tile([1, R], F32, name="sclrow")
        scl_b = const.tile([1, R], BF, name="sclb")
        rs_row = const.tile([1, R], F32, name="rsrow")
        sn_row = const.tile([1, R], F32, name="snrow")

        # ---- early phase: xx, Q chain, distances (scoped SBUF) -------------
        with tc.tile_pool(name="early", bufs=1) as early, \
             tc.tile_pool(name="sqp", bufs=3) as sq_pool, \
             tc.tile_pool(name="ttp", bufs=2) as tt_pool, \
             tc.tile_pool(name="clp", bufs=3) as clamp_pool, \
             tc.tile_pool(name="dsp", bufs=2) as dist_pool:

            TcT, HcT = [], []
            for k in range(KT):
                tct_t = early.tile([P, R], BF, name=f"tct{k}")
                nc.sync.dma_start(tct_t, io["TcTb"][k * P:(k + 1) * P, :])
                TcT.append(tct_t)
                hct_t = early.tile([P, R], BF, name=f"hct{k}")
                nc.sync.dma_start(hct_t, io["HcTb"][k * P:(k + 1) * P, :])
                HcT.append(hct_t)
            WqT, Wk = [], []
            for m in range(KT):
                wqt_t = early.tile([P, D], BF, name=f"wqt{m}")
                nc.sync.dma_start(wqt_t, io["WqTb"][m * P:(m + 1) * P, :])
                WqT.append(wqt_t)
                wk_t = early.tile([P, D], BF, name=f"wk{m}")
                nc.sync.dma_start(wk_t, io["Wkb"][m * P:(m + 1) * P, :])
                Wk.append(wk_t)
            bq_sb = []
            for m in range(KT):
                b_t = early.tile([P, 1], F32, name=f"bq{m}")
                nc.sync.dma_start(b_t, io["bqf"][m * P:(m + 1) * P, :])
                bq_sb.append(b_t)

            aug_rhs = early.tile([2, N], BF, name="augrhs")  # r0: 1, r1: -xx_j/2
            aug_lhs = early.tile([2, R], BF, name="auglhs")  # r0: -xx_i/2, r1: 1
            nc.vector.memset(aug_rhs[0:1, :], 1.0)
            # ALU writes must start at partition 0; row 1 is filled via DMA.
            for t_ in range(R // P):
                nc.sync.dma_start(aug_lhs[1:2, t_ * P:(t_ + 1) * P], ones_b1)

            # xx = |T_j|^2 over all rows, assembled in DRAM then DMA'd to
            # partition 1 of aug_rhs.
            xx_dram = dram.tile([1, N], BF, name="xxdram")
            for jc in range(NJC):
                ps_xx = psum.tile([1, CH], F32, tag="rowps", name="psxx")
                for k in range(KT):
                    tt_t = tt_pool.tile([P, CH], BF, tag=f"tt{k}", name=f"ttx{k}")
                    nc.sync.dma_start(
                        tt_t, io["TTb"][k * P:(k + 1) * P, jc * CH:(jc + 1) * CH])
                    sq = sq_pool.tile([P, CH], BF, tag="sq", name="sq")
                    nc.vector.tensor_mul(sq, tt_t, tt_t)
                    nc.tensor.matmul(ps_xx, ones_p, sq, start=(k == 0),
                                     stop=(k == KT - 1))
                xst = sq_pool.tile([1, CH], BF, tag="xst", bufs=2, name="xst")
                nc.vector.tensor_scalar(xst, ps_xx, -0.5, None, op0=ALU.mult)
                nc.sync.dma_start(xx_dram[0:1, jc * CH:(jc + 1) * CH], xst)
            nc.sync.dma_start(aug_rhs[1:2, :], xx_dram)
            for ic in range(NIC):
                ps_xxc = psum.tile([1, CH], F32, tag="rowps", name="psxxc")
                for k in range(KT):
                    sqc = sq_pool.tile([P, CH], BF, tag="sq", name="sqc")
                    nc.vector.tensor_mul(sqc, TcT[k][:, ic * CH:(ic + 1) * CH],
                                         TcT[k][:, ic * CH:(ic + 1) * CH])
                    nc.tensor.matmul(ps_xxc, ones_p, sqc, start=(k == 0),
                                     stop=(k == KT - 1))
                nc.vector.tensor_scalar(aug_lhs[0:1, ic * CH:(ic + 1) * CH],
                                        ps_xxc, -0.5, None, op0=ALU.mult)

            # Q chain: Q2^T = (Wk^T (Wq Hc^T + bq)) / sqrt(d)
            QT = [early.tile([P, R], BF, name=f"qt{m}") for m in range(KT)]
            for m in range(KT):
                for ic in range(NIC):
                    ps_q = psum.tile([P, CH], F32, tag="mm", name="psq")
                    for d_ in range(KT):
                        nc.tensor.matmul(ps_q, WqT[d_][:, m * P:(m + 1) * P],
                                         HcT[d_][:, ic * CH:(ic + 1) * CH],
                                         start=(d_ == 0), stop=(d_ == KT - 1))
                    nc.scalar.activation(QT[m][:, ic * CH:(ic + 1) * CH], ps_q,
                                         AF.Identity, bias=bq_sb[m])
            for k in range(KT):
                for ic in range(NIC):
                    ps_q2 = psum.tile([P, CH], F32, tag="mm", name="psq2")
                    for m in range(KT):
                        nc.tensor.matmul(ps_q2, Wk[m][:, k * P:(k + 1) * P],
                                         QT[m][:, ic * CH:(ic + 1) * CH],
                                         start=(m == 0), stop=(m == KT - 1))
                    nc.scalar.activation(Q2T[k][:, ic * CH:(ic + 1) * CH], ps_q2,
                                         AF.Copy, scale=INV_SQRT_D)

            # distances, row means, scale
            dsum = [early.tile([P, NJC], F32, name=f"dsum{it}")
                    for it in range(NIT)]
            for jc in range(NJC):
                tts = []
                for k in range(KT):
                    tt_t = tt_pool.tile([P, CH], BF, tag=f"tt{k}", name=f"ttd{k}")
                    nc.sync.dma_start(
                        tt_t, io["TTb"][k * P:(k + 1) * P, jc * CH:(jc + 1) * CH])
                    tts.append(tt_t)
                for it in range(NIT):
                    ps_d2 = psum.tile([P, CH], F32, tag="mm", name="psd2")
                    for k in range(KT):
                        nc.tensor.matmul(ps_d2, TcT[k][:, it * P:(it + 1) * P],
                                         tts[k], start=(k == 0), stop=False)
                    nc.tensor.matmul(ps_d2, aug_lhs[:, it * P:(it + 1) * P],
                                     aug_rhs[:, jc * CH:(jc + 1) * CH],
                                     start=False, stop=True)
                    t_cl = clamp_pool.tile([P, CH], BF, tag="clamp", name="tcl")
                    nc.vector.tensor_scalar(t_cl, ps_d2, -2.0, 0.0, op0=ALU.mult,
                                            op1=ALU.max)
                    dist_t = dist_pool.tile([P, CH], BF, tag="dist", name="distt")
                    nc.scalar.activation(dist_t, t_cl, AF.Sqrt, bias=eps_col,
                                         accum_out=dsum[it][:, jc:jc + 1])
            scl_dram = dram.tile([R, 1], F32, name="scldram")
            for it in range(NIT):
                red = early.tile([P, 1], F32, name=f"red{it}")
                nc.vector.reduce_sum(red, dsum[it], axis=mybir.AxisListType.X)
                tmp = early.tile([P, 1], F32, name=f"sctmp{it}")
                nc.vector.tensor_scalar(tmp, red, 1.0 / N, 1.0, op0=ALU.mult,
                                        op1=ALU.add)
                scol = early.tile([P, 1], F32, name=f"scol{it}")
                nc.vector.reciprocal(scol, tmp)
                nc.sync.dma_start(scl_dram[it * P:(it + 1) * P, :], scol)
            nc.sync.dma_start(scl_row,
                              scl_dram.rearrange("(a p) c -> a (p c)", a=1))
            nc.vector.tensor_copy(scl_b, scl_row)

        # ---- attention passes (logits -> exp -> G^T, rowsum) ---------------
        e_pool = ctx.enter_context(tc.tile_pool(name="ep", bufs=3))
        h_pool = ctx.enter_context(tc.tile_pool(name="hp", bufs=4))
        o_pool = ctx.enter_context(tc.tile_pool(name="op", bufs=2))
        for ic in range(NIC):
            g_ps = [psum.tile([P, CH], F32, tag=f"g{d_}", name=f"gps{d_}")
                    for d_ in range(KT)]
            rs_ps = psum.tile([1, CH], F32, tag="rowps", name="rsps")
            for jt in range(NJT):
                st = psum.tile([P, CH], F32, tag="mm", name="st")
                for k in range(KT):
                    nc.tensor.matmul(st, HT[k][:, jt * P:(jt + 1) * P],
                                     Q2T[k][:, ic * CH:(ic + 1) * CH],
                                     start=(k == 0), stop=(k == KT - 1))
                e_t = e_pool.tile([P, CH], BF, tag="e", name="et")
                nc.scalar.activation(e_t, st, AF.Exp)
                h_t = h_pool.tile([P, D], BF, tag="h", name="ht_s")
                nc.sync.dma_start(h_t, io["Hb"][jt * P:(jt + 1) * P, :])
                nc.tensor.matmul(rs_ps, ones_p, e_t, start=(jt == 0),
                                 stop=(jt == NJT - 1))
                for d_ in range(KT):
                    nc.tensor.matmul(g_ps[d_], h_t[:, d_ * P:(d_ + 1) * P], e_t,
                                     start=(jt == 0), stop=(jt == NJT - 1))
            for d_ in range(KT):
                nc.scalar.activation(GT[d_][:, ic * CH:(ic + 1) * CH], g_ps[d_],
                                     AF.Copy)
            nc.vector.tensor_copy(rs_row[0:1, ic * CH:(ic + 1) * CH], rs_ps)

        # ---- per-row normalization: SN = scale / rowsum, broadcast ---------
        nc.vector.reciprocal(sn_row, rs_row)
        nc.vector.tensor_mul(sn_row, sn_row, scl_row)
        for ic in range(NIC):
            ps_snb = psum.tile([P, CH], F32, tag="mm", name="pssnb")
            nc.tensor.matmul(ps_snb, ones_f1, sn_row[0:1, ic * CH:(ic + 1) * CH],
                             start=True, stop=True)
            nc.vector.tensor_copy(SNB[:, ic * CH:(ic + 1) * CH], ps_snb)
        for d_ in range(KT):
            for ic in range(NIC):
                nc.vector.tensor_mul(GT[d_][:, ic * CH:(ic + 1) * CH],
                                     GT[d_][:, ic * CH:(ic + 1) * CH],
                                     SNB[:, ic * CH:(ic + 1) * CH])

        # ---- Y^T = Wv Gn^T + (bv x scale) ----------------------------------
        for m in range(KT):
            for ic in range(NIC):
                ps_y = psum.tile([P, CH], F32, tag="mm", name="psy")
                for d_ in range(KT):
                    nc.tensor.matmul(ps_y, WvT[d_][:, m * P:(m + 1) * P],
                                     GT[d_][:, ic * CH:(ic + 1) * CH],
                                     start=(d_ == 0), stop=False)
                nc.tensor.matmul(ps_y, bv_row[0:1, m * P:(m + 1) * P],
                                 scl_b[0:1, ic * CH:(ic + 1) * CH],
                                 start=False, stop=True)
                nc.scalar.activation(YT[m][:, ic * CH:(ic + 1) * CH], ps_y,
                                     AF.Copy)

        # ---- out = Y Wo^T + bo  (natural layout) ---------------------------
        for it in range(NIT):
            ps_o = psum.tile([P, CH], F32, tag="mm", name="pso")
            for m in range(KT):
                nc.tensor.matmul(ps_o, YT[m][:, it * P:(it + 1) * P], WoT[m],
                                 start=(m == 0), stop=False)
            nc.tensor.matmul(ps_o, ones_b1, bo_row, start=False, stop=True)
            o_t = o_pool.tile([P, D], F32, tag="o", name="ot")
            nc.scalar.activation(o_t, ps_o, AF.Copy)
            nc.sync.dma_start(io["OUT"][it * P:(it + 1) * P, :], o_t)


_NC_CACHE = None


def _build():
    global _NC_CACHE
    if _NC_CACHE is not None:
        return _NC_CACHE
    nc = bacc.Bacc("TRN2", target_bir_lowering=False, debug=False,
                   enable_asserts=False, num_devices=NCORES)
    io = {
        "HTb": nc.dram_tensor("HTb", [D, N], BF, kind="ExternalInput").ap(),
        "Hb": nc.dram_tensor("Hb", [N, D], BF, kind="ExternalInput").ap(),
        "TTb": nc.dram_tensor("TTb", [D, N], BF, kind="ExternalInput").ap(),
        "TcTb": nc.dram_tensor("TcTb", [D, R], BF, kind="ExternalInput").ap(),
        "HcTb": nc.dram_tensor("HcTb", [D, R], BF, kind="ExternalInput").ap(),
        "WqTb": nc.dram_tensor("WqTb", [D, D], BF, kind="ExternalInput").ap(),
        "Wkb": nc.dram_tensor("Wkb", [D, D], BF, kind="ExternalInput").ap(),
        "WvTb": nc.dram_tensor("WvTb", [D, D], BF, kind="ExternalInput").ap(),
        "WoTb": nc.dram_tensor("WoTb", [D, D], BF, kind="ExternalInput").ap(),
        "bqf": nc.dram_tensor("bqf", [D, 1], F32, kind="ExternalInput").ap(),
        "bvb": nc.dram_tensor("bvb", [1, D], BF, kind="ExternalInput").ap(),
        "bob": nc.dram_tensor("bob", [1, D], BF, kind="ExternalInput").ap(),
        "OUT": nc.dram_tensor("OUT", [R, D], F32, kind="ExternalOutput").ap(),
    }
    with tile.TileContext(nc) as tc:
        _emit(tc, io)
    nc.compile()
    _NC_CACHE = nc
    return nc


LAST_RESULTS = None


def kernel(H, T, Wq, bq, Wk, bk, Wv, bv, Wo, bo):
    global LAST_RESULTS
    H = np.ascontiguousarray(np.asarray(H, np.float32))
    T = np.ascontiguousarray(np.asarray(T, np.float32))

    HTb = np.ascontiguousarray(H.T).astype(bf16)
    Hb = H.astype(bf16)
    TTb = np.ascontiguousarray(T.T).astype(bf16)
    shared = {
        "HTb": HTb,
        "Hb": Hb,
        "TTb": TTb,
        "WqTb": np.ascontiguousarray(np.asarray(Wq, np.float32).T).astype(bf16),
        "Wkb": np.ascontiguousarray(np.asarray(Wk, np.float32)).astype(bf16),
        "WvTb": np.ascontiguousarray(np.asarray(Wv, np.float32).T).astype(bf16),
        "WoTb": np.ascontiguousarray(np.asarray(Wo, np.float32).T).astype(bf16),
        "bqf": np.asarray(bq, np.float32).reshape(D, 1).copy(),
        "bvb": np.asarray(bv, np.float32).reshape(1, D).astype(bf16),
        "bob": np.asarray(bo, np.float32).reshape(1, D).astype(bf16),
    }
    in_maps = []
    for c in range(NCORES):
        m = dict(shared)
        m["TcTb"] = np.ascontiguousarray(TTb[:, c * R:(c + 1) * R])
        m["HcTb"] = np.ascontiguousarray(HTb[:, c * R:(c + 1) * R])
        in_maps.append(m)

    nc = _build()
    res = bass_utils.run_bass_kernel_spmd(nc, in_maps, core_ids=list(range(NCORES)))
    LAST_RESULTS = res
    out = np.concatenate([res.results[c]["OUT"] for c in range(NCORES)], axis=0)
    return np.ascontiguousarray(out.astype(np.float32))


# revision 10
# speedup vs baseline: 1.6432x; 1.6432x over previous
"""Trainium2 Bass kernel for a causal-attention-like module.

Math (reassociated from the reference nn.Module):
    dist[i,j] = sqrt(max(|T_i|^2 + |T_j|^2 - 2 T_i.T_j, 0) + 1e-8)
    scale_i   = 1 / (1 + mean_j dist[i,j])
    Q2        = (H Wq^T + bq) Wk / sqrt(d)         # bk cancels inside softmax
    E[i,j]    = exp(Q2[i,:] . H[j,:])              # logits bounded ~[-10,10]
    G         = E @ H                              # unnormalized
    out       = ((G / rowsum(E)) Wv^T + bv) * scale @ Wo^T + bo

Sharding: rows of H/T (i dimension) split across 8 cores, 1024 rows each;
H (both orientations) and the small dim x dim weights replicated.

Performance shape (measured on HW): a matmul whose PSUM bank differs from
the previous matmul's issues every N cycles (216 ns at N=512 bf16); a
same-bank successor serializes at ~379 ns. So every inner loop below is
arranged to alternate PSUM banks between consecutive matmuls:
  - distance phase: groups of 4 j-chunks accumulate in 4 rotating banks;
  - attention phase: the k-accumulation of logits for step jt is
    interleaved with the G/rowsum matmuls of step jt-1;
  - small projection chains are emitted pairwise (alternating chunks).
All large matmuls run in bf16 (full PE rate) with f32 PSUM accumulation.
"""

import math
import os
import sys

import numpy as np

for _p in ("/opt/trn_rl_repo", "/root/.axon_site", "/root/.axon_site/_ro/trn_rl_repo"):
    if os.path.isdir(_p) and _p not in sys.path:
        sys.path.append(_p)

import ml_dtypes

import concourse.bass as bass
import concourse.mybir as mybir
import concourse.tile as tile
from concourse import bacc, bass_utils

N = 8192          # total rows
D = 512           # feature dim
NCORES = 8
R = N // NCORES   # rows per core (1024)
P = 128           # partitions
KT = D // P       # 4 contraction tiles
CH = 512          # free-dim chunk (one PSUM bank of f32)
NJC = N // CH     # 16 j-chunks
NJT = N // P      # 64 j-tiles
NIC = R // CH     # 2 i-chunks
NIT = R // P      # 8 i-tiles
JG = 4            # j-chunks per distance group (= rotating PSUM banks)
BF = mybir.dt.bfloat16
F32 = mybir.dt.float32
AF = mybir.ActivationFunctionType
ALU = mybir.AluOpType
INV_SQRT_D = 1.0 / math.sqrt(D)

bf16 = ml_dtypes.bfloat16


def _emit(tc, io):
    nc = tc.nc
    from contextlib import ExitStack

    with ExitStack() as ctx:
        const = ctx.enter_context(tc.tile_pool(name="const", bufs=1))
        psum = ctx.enter_context(tc.tile_pool(name="psum", bufs=1, space="PSUM"))
        dram = ctx.enter_context(tc.tile_pool(name="dram", bufs=1, space="DRAM"))

        # ---- small shared constants ----------------------------------------
        ones_p = const.tile([P, 1], BF, name="onesp")
        nc.vector.memset(ones_p, 1.0)
        ones_f1 = const.tile([1, P], F32, name="onesf1")
        nc.vector.memset(ones_f1, 1.0)
        ones_b1 = const.tile([1, P], BF, name="onesb1")
        nc.vector.memset(ones_b1, 1.0)
        eps_col = const.tile([P, 1], F32, name="epscol")
        nc.vector.memset(eps_col, 1e-8)

        # ---- long-lived tensors (written early, read late) -----------------
        Q2T = [const.tile([P, R], BF, name=f"q2t{k}") for k in range(KT)]
        GT = [const.tile([P, R], BF, name=f"gt{d_}") for d_ in range(KT)]
        YT = [const.tile([P, R], BF, name=f"yt{m}") for m in range(KT)]
        SNB = const.tile([P, R], F32, name="snb")
        scl_row = const.tile([1, R], F32, name="sclrow")
        scl_b = const.tile([1, R], BF, name="sclb")
        rs_row = const.tile([1, R], F32, name="rsrow")
        sn_row = const.tile([1, R], F32, name="snrow")
        bv_row = const.tile([1, D], BF, name="bvrow")
        nc.sync.dma_start(bv_row, io["bvb"][:, :])
        bo_row = const.tile([1, D], BF, name="borow")
        nc.sync.dma_start(bo_row, io["bob"][:, :])
        WvT, WoT = [], []
        for m in range(KT):
            wvt_t = const.tile([P, D], BF, name=f"wvt{m}")
            nc.sync.dma_start(wvt_t, io["WvTb"][m * P:(m + 1) * P, :])
            WvT.append(wvt_t)
            wot_t = const.tile([P, D], BF, name=f"wot{m}")
            nc.sync.dma_start(wot_t, io["WoTb"][m * P:(m + 1) * P, :])
            WoT.append(wot_t)

        # ---- early phases (scoped SBUF) ------------------------------------
        with tc.tile_pool(name="early", bufs=1) as early:
            TcT = []
            for k in range(KT):
                tct_t = early.tile([P, R], BF, name=f"tct{k}")
                nc.sync.dma_start(tct_t, io["TcTb"][k * P:(k + 1) * P, :])
                TcT.append(tct_t)

            aug_rhs = early.tile([2, N], BF, name="augrhs")  # r0: 1, r1: -xx_j/2
            aug_lhs = early.tile([2, R], BF, name="auglhs")  # r0: -xx_i/2, r1: 1
            nc.vector.memset(aug_rhs[0:1, :], 1.0)
            # ALU writes must start at partition 0; row 1 is filled via DMA.
            for t_ in range(NIT):
                nc.sync.dma_start(aug_lhs[1:2, t_ * P:(t_ + 1) * P], ones_b1)

            # -- xx = |T_j|^2 over all rows; two j-chunk chains in flight ----
            xx_dram = dram.tile([1, N], BF, name="xxdram")
            with tc.tile_pool(name="xttp", bufs=2) as xtt_pool, \
                 tc.tile_pool(name="sqp", bufs=4) as sq_pool:
                for jc0 in range(0, NJC, 2):
                    pss = [psum.tile([1, CH], F32, tag="mm", bufs=3, name="psxx")
                           for _ in range(2)]
                    sqs = []
                    for u in range(2):
                        jc = jc0 + u
                        row = []
                        for k in range(KT):
                            tt_t = xtt_pool.tile([P, CH], BF, tag=f"xtt{k}{u}",
                                                 name=f"ttx{k}")
                            nc.sync.dma_start(
                                tt_t,
                                io["TTb"][k * P:(k + 1) * P, jc * CH:(jc + 1) * CH])
                            sq = sq_pool.tile([P, CH], BF, tag=f"sq{u}", name="sq")
                            nc.vector.tensor_mul(sq, tt_t, tt_t)
                            row.append(sq)
                        sqs.append(row)
                    for k in range(KT):
                        for u in range(2):
                            nc.tensor.matmul(pss[u], ones_p, sqs[u][k],
                                             start=(k == 0), stop=(k == KT - 1))
                    for u in range(2):
                        jc = jc0 + u
                        xst = sq_pool.tile([1, CH], BF, tag="xst", bufs=2,
                                           name="xst")
                        nc.vector.tensor_scalar(xst, pss[u], -0.5, None,
                                                op0=ALU.mult)
                        nc.sync.dma_start(xx_dram[0:1, jc * CH:(jc + 1) * CH],
                                          xst)
                nc.sync.dma_start(aug_rhs[1:2, :], xx_dram)
                # xx over this core's own rows -> aug_lhs row 0
                pssc = [psum.tile([1, CH], F32, tag="mm", bufs=3, name="psxxc")
                        for _ in range(NIC)]
                sqcs = [[None] * KT for _ in range(NIC)]
                for ic in range(NIC):
                    for k in range(KT):
                        sqc = sq_pool.tile([P, CH], BF, tag=f"sq{ic}",
                                           name="sqc")
                        nc.vector.tensor_mul(
                            sqc, TcT[k][:, ic * CH:(ic + 1) * CH],
                            TcT[k][:, ic * CH:(ic + 1) * CH])
                        sqcs[ic][k] = sqc
                for k in range(KT):
                    for ic in range(NIC):
                        nc.tensor.matmul(pssc[ic], ones_p, sqcs[ic][k],
                                         start=(k == 0), stop=(k == KT - 1))
                for ic in range(NIC):
                    nc.vector.tensor_scalar(
                        aug_lhs[0:1, ic * CH:(ic + 1) * CH], pssc[ic],
                        -0.5, None, op0=ALU.mult)

            # -- Q chain: Q2^T = (Wk^T (Wq Hc^T + bq)) / sqrt(d) -------------
            with tc.tile_pool(name="qpool", bufs=1) as qpool:
                HcT, WqT, Wk = [], [], []
                for k in range(KT):
                    hct_t = qpool.tile([P, R], BF, name=f"hct{k}")
                    nc.sync.dma_start(hct_t, io["HcTb"][k * P:(k + 1) * P, :])
                    HcT.append(hct_t)
                    wqt_t = qpool.tile([P, D], BF, name=f"wqt{k}")
                    nc.sync.dma_start(wqt_t, io["WqTb"][k * P:(k + 1) * P, :])
                    WqT.append(wqt_t)
                    wk_t = qpool.tile([P, D], BF, name=f"wk{k}")
                    nc.sync.dma_start(wk_t, io["Wkb"][k * P:(k + 1) * P, :])
                    Wk.append(wk_t)
                bq_sb = []
                for m in range(KT):
                    b_t = qpool.tile([P, 1], F32, name=f"bq{m}")
                    nc.sync.dma_start(b_t, io["bqf"][m * P:(m + 1) * P, :])
                    bq_sb.append(b_t)
                QT = [qpool.tile([P, R], BF, name=f"qt{m}") for m in range(KT)]
                for m in range(KT):
                    pq = [psum.tile([P, CH], F32, tag="mm", bufs=3, name="psq")
                          for _ in range(NIC)]
                    for d_ in range(KT):
                        for ic in range(NIC):
                            nc.tensor.matmul(pq[ic],
                                             WqT[d_][:, m * P:(m + 1) * P],
                                             HcT[d_][:, ic * CH:(ic + 1) * CH],
                                             start=(d_ == 0),
                                             stop=(d_ == KT - 1))
                    for ic in range(NIC):
                        nc.scalar.activation(QT[m][:, ic * CH:(ic + 1) * CH],
                                             pq[ic], AF.Identity, bias=bq_sb[m])
                for k in range(KT):
                    pq2 = [psum.tile([P, CH], F32, tag="mm", bufs=3, name="psq2")
                           for _ in range(NIC)]
                    for m in range(KT):
                        for ic in range(NIC):
                            nc.tensor.matmul(pq2[ic],
                                             Wk[m][:, k * P:(k + 1) * P],
                                             QT[m][:, ic * CH:(ic + 1) * CH],
                                             start=(m == 0), stop=(m == KT - 1))
                    for ic in range(NIC):
                        nc.scalar.activation(Q2T[k][:, ic * CH:(ic + 1) * CH],
                                             pq2[ic], AF.Copy,
                                             scale=INV_SQRT_D)

            # -- distances: 4 j-chunks per group in 4 rotating banks ---------
            dsum = [early.tile([P, NJC], F32, name=f"dsum{it}")
                    for it in range(NIT)]
            with tc.tile_pool(name="ttp", bufs=2) as tt_pool, \
                 tc.tile_pool(name="clp", bufs=4) as clamp_pool, \
                 tc.tile_pool(name="dsp", bufs=4) as dist_pool:
                for jg in range(NJC // JG):
                    tts = [[None] * JG for _ in range(KT)]
                    for jj in range(JG):
                        jc = jg * JG + jj
                        for k in range(KT):
                            tt_t = tt_pool.tile([P, CH], BF, tag=f"tt{k}{jj}",
                                                name=f"ttd{k}")
                            nc.sync.dma_start(
                                tt_t,
                                io["TTb"][k * P:(k + 1) * P,
                                          jc * CH:(jc + 1) * CH])
                            tts[k][jj] = tt_t
                    for it in range(NIT):
                        pd = [psum.tile([P, CH], F32, tag=f"g{jj}",
                                        name=f"psd{jj}") for jj in range(JG)]
                        for k in range(KT):
                            for jj in range(JG):
                                nc.tensor.matmul(
                                    pd[jj], TcT[k][:, it * P:(it + 1) * P],
                                    tts[k][jj], start=(k == 0), stop=False)
                        for jj in range(JG):
                            jc = jg * JG + jj
                            nc.tensor.matmul(
                                pd[jj], aug_lhs[:, it * P:(it + 1) * P],
                                aug_rhs[:, jc * CH:(jc + 1) * CH],
                                start=False, stop=True)
                        for jj in range(JG):
                            jc = jg * JG + jj
                            t_cl = clamp_pool.tile([P, CH], BF, tag="clamp",
                                                   name="tcl")
                            nc.vector.tensor_scalar(t_cl, pd[jj], -2.0, 0.0,
                                                    op0=ALU.mult, op1=ALU.max)
                            dist_t = dist_pool.tile([P, CH], BF, tag="dist",
                                                    name="distt")
                            nc.scalar.activation(
                                dist_t, t_cl, AF.Sqrt, bias=eps_col,
                                accum_out=dsum[it][:, jc:jc + 1])
            scl_dram = dram.tile([R, 1], F32, name="scldram")
            for it in range(NIT):
                red = early.tile([P, 1], F32, name=f"red{it}")
                nc.vector.reduce_sum(red, dsum[it], axis=mybir.AxisListType.X)
                tmp = early.tile([P, 1], F32, name=f"sctmp{it}")
                nc.vector.tensor_scalar(tmp, red, 1.0 / N, 1.0, op0=ALU.mult,
                                        op1=ALU.add)
                scol = early.tile([P, 1], F32, name=f"scol{it}")
                nc.vector.reciprocal(scol, tmp)
                nc.sync.dma_start(scl_dram[it * P:(it + 1) * P, :], scol)
            nc.sync.dma_start(scl_row,
                              scl_dram.rearrange("(a p) c -> a (p c)", a=1))
            nc.vector.tensor_copy(scl_b, scl_row)

        # ---- resident transposed H (consumed by the attention passes) ------
        HT = []
        for k in range(KT):
            ht_t = const.tile([P, N], BF, name=f"ht{k}")
            nc.sync.dma_start(ht_t, io["HTb"][k * P:(k + 1) * P, :])
            HT.append(ht_t)

        # ---- attention passes: pipelined logits(jt) | G/rowsum(jt-1) -------
        e_pool = ctx.enter_context(tc.tile_pool(name="ep", bufs=4))
        h_pool = ctx.enter_context(tc.tile_pool(name="hp", bufs=8))
        o_pool = ctx.enter_context(tc.tile_pool(name="op", bufs=2))
        for ic in range(NIC):
            csl = slice(ic * CH, (ic + 1) * CH)
            g_ps = [psum.tile([P, CH], F32, tag=f"g{d_}", name=f"gps{d_}")
                    for d_ in range(KT)]
            rs_ps = psum.tile([1, CH], F32, tag="rowps", name="rsps")
            prev_e = prev_h = None
            prev_jt = -1
            for jt in range(NJT):
                h_t = h_pool.tile([P, D], BF, tag="h", name="ht_s")
                nc.sync.dma_start(h_t, io["Hb"][jt * P:(jt + 1) * P, :])
                st = psum.tile([P, CH], F32, tag="mm", bufs=3, name="st")
                for k in range(KT):
                    nc.tensor.matmul(st, HT[k][:, jt * P:(jt + 1) * P],
                                     Q2T[k][:, csl],
                                     start=(k == 0), stop=(k == KT - 1))
                    if prev_e is not None:
                        nc.tensor.matmul(g_ps[k],
                                         prev_h[:, k * P:(k + 1) * P], prev_e,
                                         start=(prev_jt == 0),
                                         stop=(prev_jt == NJT - 1))
                if prev_e is not None:
                    nc.tensor.matmul(rs_ps, ones_p, prev_e,
                                     start=(prev_jt == 0),
                                     stop=(prev_jt == NJT - 1))
                e_t = e_pool.tile([P, CH], BF, tag="e", name="et")
                nc.scalar.activation(e_t, st, AF.Exp)
                prev_e, prev_h, prev_jt = e_t, h_t, jt
            for k in range(KT):
                nc.tensor.matmul(g_ps[k], prev_h[:, k * P:(k + 1) * P], prev_e,
                                 start=(prev_jt == 0), stop=True)
            nc.tensor.matmul(rs_ps, ones_p, prev_e,
                             start=(prev_jt == 0), stop=True)

            # ---- per-chunk tail: SN, normalize, Y^T, output ----------------
            for d_ in range(KT):
                nc.scalar.activation(GT[d_][:, csl], g_ps[d_], AF.Copy)
            nc.vector.tensor_copy(rs_row[0:1, csl], rs_ps)
            nc.vector.reciprocal(sn_row[0:1, csl], rs_row[0:1, csl])
            nc.vector.tensor_mul(sn_row[0:1, csl], sn_row[0:1, csl],
                                 scl_row[0:1, csl])
            ps_snb = psum.tile([P, CH], F32, tag="mm", bufs=3, name="pssnb")
            nc.tensor.matmul(ps_snb, ones_f1, sn_row[0:1, csl],
                             start=True, stop=True)
            nc.vector.tensor_copy(SNB[:, csl], ps_snb)
            for d_ in range(KT):
                nc.vector.tensor_mul(GT[d_][:, csl], GT[d_][:, csl],
                                     SNB[:, csl])
            # Y^T = Wv Gn^T + (bv x scale): two m-chains in flight
            for m0 in range(0, KT, 2):
                py = [psum.tile([P, CH], F32, tag="mm", bufs=3, name="psy")
                      for _ in range(2)]
                for d_ in range(KT):
                    for u in range(2):
                        m = m0 + u
                        nc.tensor.matmul(py[u], WvT[d_][:, m * P:(m + 1) * P],
                                         GT[d_][:, csl],
                                         start=(d_ == 0), stop=False)
                for u in range(2):
                    m = m0 + u
                    nc.tensor.matmul(py[u], bv_row[0:1, m * P:(m + 1) * P],
                                     scl_b[0:1, csl], start=False, stop=True)
                for u in range(2):
                    m = m0 + u
                    nc.scalar.activation(YT[m][:, csl], py[u], AF.Copy)
            # out = Y Wo^T + bo for this chunk's 4 i-tiles, chains in pairs
            for it0 in range(ic * 4, (ic + 1) * 4, 2):
                po = [psum.tile([P, CH], F32, tag="mm", bufs=3, name="pso")
                      for _ in range(2)]
                for m in range(KT):
                    for u in range(2):
                        it = it0 + u
                        nc.tensor.matmul(po[u], YT[m][:, it * P:(it + 1) * P],
                                         WoT[m], start=(m == 0), stop=False)
                for u in range(2):
                    nc.tensor.matmul(po[u], ones_b1, bo_row,
                                     start=False, stop=True)
                for u in range(2):
                    it = it0 + u
                    o_t = o_pool.tile([P, D], F32, tag="o", name="ot")
                    nc.scalar.activation(o_t, po[u], AF.Copy)
                    nc.sync.dma_start(io["OUT"][it * P:(it + 1) * P, :], o_t)


_NC_CACHE = None


def _build():
    global _NC_CACHE
    if _NC_CACHE is not None:
        return _NC_CACHE
    nc = bacc.Bacc("TRN2", target_bir_lowering=False, debug=False,
                   enable_asserts=False, num_devices=NCORES)
    io = {
        "HTb": nc.dram_tensor("HTb", [D, N], BF, kind="ExternalInput").ap(),
        "Hb": nc.dram_tensor("Hb", [N, D], BF, kind="ExternalInput").ap(),
        "TTb": nc.dram_tensor("TTb", [D, N], BF, kind="ExternalInput").ap(),
        "TcTb": nc.dram_tensor("TcTb", [D, R], BF, kind="ExternalInput").ap(),
        "HcTb": nc.dram_tensor("HcTb", [D, R], BF, kind="ExternalInput").ap(),
        "WqTb": nc.dram_tensor("WqTb", [D, D], BF, kind="ExternalInput").ap(),
        "Wkb": nc.dram_tensor("Wkb", [D, D], BF, kind="ExternalInput").ap(),
        "WvTb": nc.dram_tensor("WvTb", [D, D], BF, kind="ExternalInput").ap(),
        "WoTb": nc.dram_tensor("WoTb", [D, D], BF, kind="ExternalInput").ap(),
        "bqf": nc.dram_tensor("bqf", [D, 1], F32, kind="ExternalInput").ap(),
        "bvb": nc.dram_tensor("bvb", [1, D], BF, kind="ExternalInput").ap(),
        "bob": nc.dram_tensor("bob", [1, D], BF, kind="ExternalInput").ap(),
        "OUT": nc.dram_tensor("OUT", [R, D], F32, kind="ExternalOutput").ap(),
    }
    with tile.TileContext(nc) as tc:
        _emit(tc, io)
    nc.compile()
    _NC_CACHE = nc
    return nc


LAST_RESULTS = None


def kernel(H, T, Wq, bq, Wk, bk, Wv, bv, Wo, bo):
    global LAST_RESULTS
    H = np.ascontiguousarray(np.asarray(H, np.float32))
    T = np.ascontiguousarray(np.asarray(T, np.float32))

    HTb = np.ascontiguousarray(H.T).astype(bf16)
    Hb = H.astype(bf16)
    TTb = np.ascontiguousarray(T.T).astype(bf16)
    shared = {
        "HTb": HTb,
        "Hb": Hb,
        "TTb": TTb,
        "WqTb": np.ascontiguousarray(np.asarray(Wq, np.float32).T).astype(bf16),
        "Wkb": np.ascontiguousarray(np.asarray(Wk, np.float32)).astype(bf16),
        "WvTb": np.ascontiguousarray(np.asarray(Wv, np.float32).T).astype(bf16),
        "WoTb": np.ascontiguousarray(np.asarray(Wo, np.float32).T).astype(bf16),
        "bqf": np.asarray(bq, np.float32).reshape(D, 1).copy(),
        "bvb": np.asarray(bv, np.float32).reshape(1, D).astype(bf16),
        "bob": np.asarray(bo, np.float32).reshape(1, D).astype(bf16),
    }
    in_maps = []
    for c in range(NCORES):
        m = dict(shared)
        m["TcTb"] = np.ascontiguousarray(TTb[:, c * R:(c + 1) * R])
        m["HcTb"] = np.ascontiguousarray(HTb[:, c * R:(c + 1) * R])
        in_maps.append(m)

    nc = _build()
    res = bass_utils.run_bass_kernel_spmd(nc, in_maps, core_ids=list(range(NCORES)))
    LAST_RESULTS = res
    out = np.concatenate([res.results[c]["OUT"] for c in range(NCORES)], axis=0)
    return np.ascontiguousarray(out.astype(np.float32))
